# revision 19
# baseline (speedup 1.0000x reference)
"""Trainium2 Bass kernel for nn_LBGC_v4 (embedding_lookup).

Computation (see reference):
    s_b   = (P[t_b] @ u_b + time[t_b]) . poi_b          per sample
    pos_b = -log_sigmoid(s_b)                            4096 positive samples
    neg_k = sum_b -log_sigmoid(-s_kb)                    10 rows x 4096 negatives

Strategy:
  * 45056 samples sharded over 8 cores by time-index t (21 of 168 t's per
    core, dealt snake-order by per-t frequency so the static per-rank group
    lengths padded to the max over cores waste only a few % of slots).
  * Within a core samples are grouped by t, so the per-sample 128x128 matvec
    becomes one matmul per t-group with stationary projT[t].
  * user/poi rows are gathered on device with indirect DMA (int32 indices).
  * W = projT[t]^T-applied u + time[t] (time-add fused into the PSUM->SBUF
    copy as a per-partition scalar add).
  * scores via PE transpose of W tiles + one scalar_tensor_tensor per tile
    (multiply with gathered poi, accumulate along free dim).
  * v = softplus(-sign*s) in one ACT instruction; per-core neg partials via
    masked scalar_tensor_tensor accumulations; host scatters pos and sums neg.
"""

import numpy as np

import concourse.bass as bass
import concourse.tile as tile
from concourse import bacc
from concourse import mybir
from concourse.bass_utils import run_bass_kernel_spmd
from concourse.masks import make_identity

NCORES = 8
T_TOT = 168
TPC = T_TOT // NCORES  # 21 t's per core
D = 128
U_TOT = 100000
P_TOT = 100000
NS = 10
B = 4096
F32 = mybir.dt.float32
I32 = mybir.dt.int32

_prog_cache = {}
TRACE = False
LINEARIZE = False
LAST_RESULT = None  # BassKernelResults of the most recent run (for profiling)


def _build_program(C, segments, chunk_tiles=1):
    """segments: list of (slab_idx, offset, length) covering [0, C*128)."""
    nc = bacc.Bacc(None, target_bir_lowering=False)
    N = C * 128

    user = nc.declare_dram_parameter("user_emb", [U_TOT, D], F32, isOutput=False)
    poit = nc.declare_dram_parameter("poi_emb", [P_TOT, D], F32, isOutput=False)
    projT = nc.declare_dram_parameter("projT", [128, TPC * 128], F32, isOutput=False)
    timeT = nc.declare_dram_parameter("timeT", [128, TPC], F32, isOutput=False)
    uidx = nc.declare_dram_parameter("uidx", [128, C], I32, isOutput=False)
    pidx = nc.declare_dram_parameter("pidx", [128, C], I32, isOutput=False)
    sgn = nc.declare_dram_parameter("sgn", [128, C], F32, isOutput=False)
    masks = nc.declare_dram_parameter("masks", [128, NS * C], F32, isOutput=False)
    vout = nc.declare_dram_parameter("vout", [128, C], F32, isOutput=True)
    negout = nc.declare_dram_parameter("negout", [NS, 1], F32, isOutput=True)

    with tile.TileContext(nc, linearize=LINEARIZE) as tc, \
            tc.tile_pool(name="persist", bufs=1) as pp:
        # ---- persistent SBUF tensors (one named slot each) ---------------
        ident = pp.tile([128, 128], F32, name="ident")
        projT_sb = pp.tile([128, TPC * 128], F32, name="projT_sb")
        timeT_sb = pp.tile([128, TPC], F32, name="timeT_sb")
        uidx_sb = pp.tile([128, C], I32, name="uidx_sb")
        pidx_sb = pp.tile([128, C], I32, name="pidx_sb")
        sgn_sb = pp.tile([128, C], F32, name="sgn_sb")
        masks_sb = pp.tile([128, NS * C], F32, name="masks_sb")
        u_g = pp.tile([128, N], F32, name="u_g")
        poi_g = pp.tile([128, N], F32, name="poi_g")
        uT = pp.tile([128, N], F32, name="uT")
        w_sb = pp.tile([128, N], F32, name="w_sb")
        scores = pp.tile([128, C], F32, name="scores")
        ss = pp.tile([128, C], F32, name="ss")
        vv = pp.tile([128, C], F32, name="vv")
        negacc = pp.tile([128, NS], F32, name="negacc")
        negp = pp.tile([NS, 1], F32, name="negp")

        make_identity(nc, ident[:])

        # ---- input loads -------------------------------------------------
        nc.sync.dma_start(out=uidx_sb[:], in_=uidx[:])
        nc.sync.dma_start(out=pidx_sb[:], in_=pidx[:])
        nc.sync.dma_start(out=sgn_sb[:], in_=sgn[:])
        nc.sync.dma_start(out=masks_sb[:], in_=masks[:])
        nc.sync.dma_start(out=timeT_sb[:], in_=timeT[:])
        nc.sync.dma_start(out=projT_sb[:], in_=projT[:])

        # ---- gathers (chunked so compute can start early) ---------------
        nchunk = (C + chunk_tiles - 1) // chunk_tiles
        for ch in range(nchunk):
            c0 = ch * chunk_tiles
            c1 = min(C, c0 + chunk_tiles)
            nc.gpsimd.indirect_dma_start(
                out=u_g[:, c0 * 128 : c1 * 128],
                out_offset=None,
                in_=user[:],
                in_offset=bass.IndirectOffsetOnAxis(ap=uidx_sb[:, c0:c1], axis=0),
            )
            nc.gpsimd.indirect_dma_start(
                out=poi_g[:, c0 * 128 : c1 * 128],
                out_offset=None,
                in_=poit[:],
                in_offset=bass.IndirectOffsetOnAxis(ap=pidx_sb[:, c0:c1], axis=0),
            )

        # ---- transpose gathered u tiles: uT[:, s] = u_g[s%128, :] -------
        # PE Matmult instructions can carry only ONE semaphore wait, so each
        # batch is preceded by a dummy transpose of the identity into the
        # recycled PSUM slot: it absorbs the slot-free dependency, leaving
        # the real transposes with just their data dependency.
        with tc.tile_pool(name="tps", bufs=3, space="PSUM") as tps:
            for b0 in range(0, C, 4):
                b1 = min(C, b0 + 4)
                pt = tps.tile([128, 512], F32)
                nc.tensor.transpose(
                    out=pt[:, :128], in_=ident[:], identity=ident[:]
                )
                for j, c in enumerate(range(b0, b1)):
                    nc.tensor.transpose(
                        out=pt[:, j * 128 : (j + 1) * 128],
                        in_=u_g[:, c * 128 : (c + 1) * 128],
                        identity=ident[:],
                    )
                # copy the batch out on the scalar engine (frees DVE)
                nc.scalar.copy(
                    out=uT[:, b0 * 128 : b1 * 128], in_=pt[:, : (b1 - b0) * 128]
                )

        # ---- per-group matmuls: W = projT[t]^T-applied u (+time) --------
        with tc.tile_pool(name="wps", bufs=3, space="PSUM") as wps:
            first = True
            for slab, off, ln in segments:
                wp = wps.tile([128, 512], F32)
                if first:
                    # absorb the projT HWDGE-load wait on a dummy so the
                    # first real matmul carries only the uT (ACT) wait
                    nc.tensor.transpose(
                        out=wp[:, :128], in_=projT_sb[:, :128], identity=ident[:]
                    )
                    first = False
                nc.tensor.matmul(
                    out=wp[:, :ln],
                    lhsT=projT_sb[:, slab * 128 : (slab + 1) * 128],
                    rhs=uT[:, off : off + ln],
                    start=True,
                    stop=True,
                )
                # fused time-add during PSUM->SBUF copy (on ACT, frees DVE)
                nc.scalar.activation(
                    out=w_sb[:, off : off + ln],
                    in_=wp[:, :ln],
                    func=mybir.ActivationFunctionType.Identity,
                    bias=timeT_sb[:, slab : slab + 1],
                )

        # ---- scores: transpose W tiles, multiply by poi, reduce ---------
        with (
            tc.tile_pool(name="wtps", bufs=3, space="PSUM") as wtps,
            tc.tile_pool(name="scr", bufs=4) as scr,
        ):
            for b0 in range(0, C, 4):
                b1 = min(C, b0 + 4)
                pt = wtps.tile([128, 512], F32)
                nc.tensor.transpose(
                    out=pt[:, :128], in_=ident[:], identity=ident[:]
                )
                for j, c in enumerate(range(b0, b1)):
                    nc.tensor.transpose(
                        out=pt[:, j * 128 : (j + 1) * 128],
                        in_=w_sb[:, c * 128 : (c + 1) * 128],
                        identity=ident[:],
                    )
                for j, c in enumerate(range(b0, b1)):
                    dst = scr.tile([128, 128], F32)
                    nc.vector.scalar_tensor_tensor(
                        out=dst[:],
                        in0=pt[:, j * 128 : (j + 1) * 128],
                        scalar=1.0,
                        in1=poi_g[:, c * 128 : (c + 1) * 128],
                        op0=mybir.AluOpType.mult,
                        op1=mybir.AluOpType.mult,
                        accum_out=scores[:, c : c + 1],
                    )

        # ---- v = softplus(-sign*s) --------------------------------------
        nc.vector.tensor_tensor(
            out=ss[:], in0=scores[:], in1=sgn_sb[:], op=mybir.AluOpType.mult
        )
        # v = softplus(-ss) = ln(1 + exp(-ss)); Exp and Ln share a table set
        nc.scalar.activation(
            out=vv[:], in_=ss[:], func=mybir.ActivationFunctionType.Exp,
            scale=-1.0,
        )
        nc.scalar.activation(
            out=vv[:], in_=vv[:], func=mybir.ActivationFunctionType.Ln,
            bias=1.0,
        )
        nc.sync.dma_start(out=vout[:], in_=vv[:])

        # ---- neg partials: masked accumulations then partition reduce ---
        with tc.tile_pool(name="nscr", bufs=4) as nscr:
            for k in range(NS):
                dst = nscr.tile([128, C], F32)
                nc.vector.scalar_tensor_tensor(
                    out=dst[:],
                    in0=vv[:],
                    scalar=1.0,
                    in1=masks_sb[:, k * C : (k + 1) * C],
                    op0=mybir.AluOpType.mult,
                    op1=mybir.AluOpType.mult,
                    accum_out=negacc[:, k : k + 1],
                )
        with tc.tile_pool(name="nps", bufs=1, space="PSUM") as nps:
            npt = nps.tile([128, 128], F32)
            nc.tensor.transpose(out=npt[:NS, :], in_=negacc[:], identity=ident[:])
            nc.vector.tensor_reduce(
                out=negp[:],
                in_=npt[:NS, :],
                axis=mybir.AxisListType.X,
                op=mybir.AluOpType.add,
            )
        nc.sync.dma_start(out=negout[:], in_=negp[:])

    return nc


def _plan(t_all):
    """Shard t values over cores; return per-core t lists + shared group lens."""
    counts = np.bincount(t_all, minlength=T_TOT)
    order = np.argsort(-counts, kind="stable")  # t ids, most frequent first
    core_ts = [[] for _ in range(NCORES)]
    for r in range(TPC):
        blk = order[r * NCORES : (r + 1) * NCORES]
        seq = range(NCORES) if r % 2 == 0 else range(NCORES - 1, -1, -1)
        for i, c in enumerate(seq):
            core_ts[c].append(blk[i])
    core_ts = np.array(core_ts)  # [8, 21]
    lens = counts[core_ts].max(axis=0)  # [21] shared rank lengths
    total = int(lens.sum())
    C = (total + 127) // 128
    pad = C * 128 - total
    lens = lens.copy()
    lens[-1] += pad  # extend last group to cover the tail exactly
    offs = np.concatenate([[0], np.cumsum(lens)])[:-1]
    segments = []
    for k in range(TPC):
        off, ln = int(offs[k]), int(lens[k])
        while ln > 0:  # PSUM bank limit: 512 fp32 per matmul
            seg = min(ln, 512)
            segments.append((k, off, seg))
            off += seg
            ln -= seg
    return core_ts, lens, offs, C, segments


def kernel(pos_u, pos_t, pos_p, neg_u, neg_t, neg_p, NS: int,  # noqa: N803
           user_emb, poi_emb, time_emb, proj_emb):
    ns = int(NS)
    assert ns == 10
    u_all = np.concatenate([np.asarray(pos_u), np.asarray(neg_u).ravel()]).astype(np.int64)
    t_all = np.concatenate([np.asarray(pos_t), np.asarray(neg_t).ravel()]).astype(np.int64)
    p_all = np.concatenate([np.asarray(pos_p), np.asarray(neg_p).ravel()]).astype(np.int64)
    nsamp = u_all.shape[0]
    b = np.asarray(pos_u).shape[0]
    # tag: -1 for positive samples, else neg row k
    tag = np.full(nsamp, -1, np.int64)
    tag[b:] = np.repeat(np.arange(ns), b)
    sign = np.where(tag < 0, 1.0, -1.0).astype(np.float32)

    core_ts, lens, offs, C, segments = _plan(t_all)
    N = C * 128

    key = (C, tuple(segments))
    if key not in _prog_cache:
        nc_new = _build_program(C, segments)
        if not nc_new.is_finalized():
            nc_new.finalize()
        _prog_cache[key] = nc_new
    nc = _prog_cache[key]

    user_np = np.ascontiguousarray(np.asarray(user_emb, np.float32))
    poi_np = np.ascontiguousarray(np.asarray(poi_emb, np.float32))
    time_np = np.asarray(time_emb, np.float32)
    proj_np = np.asarray(proj_emb, np.float32).reshape(T_TOT, D, D)

    # bucket sample ids by t for fast lookup
    t_sort = np.argsort(t_all, kind="stable")
    t_bounds = np.searchsorted(t_all[t_sort], np.arange(T_TOT + 1))

    in_maps = []
    slot_sample = np.full((NCORES, N), -1, np.int64)  # global sample id per slot
    for c in range(NCORES):
        ts = core_ts[c]
        uidx = np.zeros(N, np.int32)
        pidx = np.zeros(N, np.int32)
        sg = np.zeros(N, np.float32)
        mk = np.zeros((ns, N), np.float32)
        for k in range(TPC):
            t = ts[k]
            sids = t_sort[t_bounds[t] : t_bounds[t + 1]]
            off = int(offs[k])
            sl = slice(off, off + sids.shape[0])
            uidx[sl] = u_all[sids]
            pidx[sl] = p_all[sids]
            sg[sl] = sign[sids]
            slot_sample[c, sl] = sids
            kk = tag[sids]
            isneg = kk >= 0
            mk[kk[isneg], off + np.nonzero(isneg)[0]] = 1.0
        # device layout [128, C]: dev[p, cc] = arr[cc*128 + p]
        dev = lambda a: np.ascontiguousarray(a.reshape(C, 128).T)
        projT = np.ascontiguousarray(
            proj_np[ts].transpose(2, 0, 1).reshape(128, TPC * 128)
        )
        timeT = np.ascontiguousarray(time_np[ts].T)
        mkdev = np.ascontiguousarray(
            mk.reshape(ns, C, 128).transpose(2, 0, 1).reshape(128, ns * C)
        )
        in_maps.append(
            dict(
                user_emb=user_np,
                poi_emb=poi_np,
                projT=projT,
                timeT=timeT,
                uidx=dev(uidx),
                pidx=dev(pidx),
                sgn=dev(sg),
                masks=mkdev,
            )
        )

    global LAST_RESULT
    LAST_RESULT = run_bass_kernel_spmd(
        nc, in_maps, list(range(NCORES)), trace=TRACE
    )
    res = LAST_RESULT.results

    pos = np.zeros(b, np.float32)
    neg = np.zeros(ns, np.float32)
    all_vals = []
    for c in range(NCORES):
        vals = np.asarray(res[c]["vout"]).T.ravel()  # slot-ordered values
        all_vals.append(vals)
        neg += np.asarray(res[c]["negout"]).ravel()
        sm = slot_sample[c]
        is_pos = (sm >= 0) & (sm < b)
        pos[sm[is_pos]] = vals[is_pos]
    global LAST_DEBUG
    LAST_DEBUG = dict(slot_sample=slot_sample, vals=all_vals, core_ts=core_ts,
                      offs=offs, lens=lens, C=C, res=res, in_maps=in_maps,
                      u_all=u_all, t_all=t_all, p_all=p_all, sign=sign)
    return pos, neg
